# revision 9
# baseline (speedup 1.0000x reference)
"""Trainium2 kernel for the e3nn-style equivariant 3D convolution.

Math: out = self_connection(x) + conv3d(x, K) where K[32,32,5,5,5] is
generated from tiny weights. The self-connection is a channel-mixing 1x1x1
op, folded into the conv kernel's center tap, so the whole module is ONE
5^3 conv with 32 in/out channels over x[2, 32, 64, 64, 64].

Distribution: 8 cores = 2 batches x 4 X-chunks of 16 rows (data parallel
with 2-row halos; halos come from host-side padding, no collectives).

Per-core compute scheme (PE-array-shaped):
- SBUF holds, per input x-row, a [128, 4880] tile: partition (a*32 + c) =
  channel c of z-shift-copy a (a = 0..3), free dim = padded (y,z) plane
  (68*68 = 4624) with 128-elem guard bands.
- The 125 taps are covered by 15 matmuls per 512-wide output tile:
  contraction = 128 (32 ch x 4 z-copies), output = 128 (32 ch x 4
  "output shift" groups W = {(0,0),(1,0),(0,2),(1,2)} in (dy,dz)).
  Instruction (dx, ty) reads row (r+dx) at in-plane offset ty*68; block
  (a, b) then realizes tap (dx, ty - w_b.y, a - w_b.z). Each of the 125
  taps is assigned to exactly one block; unused blocks hold zero weight.
- Drain (all ops partition-aligned; TRN2 DVE/ACT cannot cross partitions):
  psum group b is copied to gbuf at free offset +w_b_flat (same partitions),
  then a constant [128,32] stacked-identity "collapse" matmul sums the four
  groups: out[f] = sum_b g_b[f - w_b].
- dtype: float32r (TF32-like matmul, ~1.5e-4 rel err, ~4x faster than
  fp32 on the PE array). PSUM accumulates in fp32.
"""

import math

import numpy as np

MUL = 8
C = 4 * MUL          # 32 channels
NROW = 16            # output x-rows per core
YP = 68              # padded y/z plane edge
S = YP * YP          # 4624 plane elems
G = 128              # guard band elems
TOT = G + S + G      # xin tile width
TILEN = 512
NTILE = 9            # 9*512 = 4608 >= needed 4488
W_SET = ((0, 0), (1, 0), (0, 2), (1, 2))   # output-shift groups (dy, dz)
TY_SET = (-1, 1, 2)
INSTRS = tuple((dx, ty) for dx in range(-2, 3) for ty in TY_SET)
NI = len(INSTRS)     # 15


# ---------------------------------------------------------------------------
# Host-side math: build the folded conv kernel (numpy port of the reference)
# ---------------------------------------------------------------------------

def _sus(t):
    return np.where(t > 0.0, np.exp(-1.0 / np.where(t > 0.0, t, 1.0)), 0.0)


def _emb_sh():
    c = np.arange(-2.0, 3.0)
    lat = np.stack(np.meshgrid(c, c, c, indexing="ij"), axis=-1)
    d = np.linalg.norm(lat, axis=-1)
    NB, RAD = 5, 2.5
    values = np.linspace(0.0, RAD, NB + 2)
    step = values[1] - values[0]
    diff = (d[..., None] - values[1:-1]) / step
    emb = 1.14136 * math.exp(2.0) * _sus(diff + 1.0) * _sus(1.0 - diff)
    safe = np.where(d > 0.0, d, 1.0)[..., None]
    u = np.where(d[..., None] > 0.0, lat / safe, 0.0)
    x, y, z = u[..., 0], u[..., 1], u[..., 2]
    s3, s5, s15 = math.sqrt(3.0), math.sqrt(5.0), math.sqrt(15.0)
    sh = np.stack([
        np.ones_like(x),
        s3 * x, s3 * y, s3 * z,
        s15 * x * z, s15 * x * y,
        s5 * (y * y - 0.5 * (x * x + z * z)),
        s15 * y * z, (s15 / 2.0) * (z * z - x * x),
    ], axis=-1)
    return emb, sh


def _c121():
    s2, s6 = math.sqrt(2.0), math.sqrt(6.0)
    Sm = np.zeros((5, 3, 3))
    Sm[0, 0, 2] = Sm[0, 2, 0] = 1.0 / s2
    Sm[1, 0, 1] = Sm[1, 1, 0] = 1.0 / s2
    Sm[2] = np.diag([-1.0, 2.0, -1.0]) / s6
    Sm[3, 1, 2] = Sm[3, 2, 1] = 1.0 / s2
    Sm[4] = np.diag([-1.0, 0.0, 1.0]) / s2
    return np.transpose(Sm, (1, 0, 2)) / math.sqrt(5.0)


def build_folded_kernel(weight, w_sc0, w_sc1):
    """Returns kern[o, i, kx, ky, kz] float32 with self-connection folded."""
    weight = weight.astype(np.float64)
    emb, sh = _emb_sh()
    c121 = _c121()
    g = (5, 5, 5)
    W = (emb @ weight) / 125.0
    w1, w2, w3, w4, w5 = [
        W[..., 64 * i:64 * (i + 1)].reshape(g + (MUL, MUL)) for i in range(5)
    ]
    sh0, sh1, sh2 = sh[..., 0], sh[..., 1:4], sh[..., 4:9]
    pw0 = math.sqrt(1.0 / 16.0)
    pw1 = math.sqrt(3.0 / 24.0)
    inv_s3 = 1.0 / math.sqrt(3.0)
    K00 = pw0 * sh0[..., None, None] * w1
    K01 = (pw1 * inv_s3) * np.einsum("xyzuw,xyzk->xyzuwk", w2, sh1)
    K10 = (pw0 * inv_s3) * np.einsum("xyzuw,xyzm->xyzumw", w4, sh1)
    T = np.einsum("xyzj,mjk->xyzmk", sh2, c121)
    eye3 = np.eye(3)
    K11 = pw1 * (
        inv_s3 * np.einsum("xyzuw,mk->xyzumwk", sh0[..., None, None] * w3, eye3)
        + np.einsum("xyzuw,xyzmk->xyzumwk", w5, T)
    )
    D = 4 * MUL
    kern = np.zeros(g + (D, D))
    kern[..., :MUL, :MUL] = K00
    kern[..., :MUL, MUL:] = K01.reshape(g + (MUL, 3 * MUL))
    kern[..., MUL:, :MUL] = K10.reshape(g + (3 * MUL, MUL))
    kern[..., MUL:, MUL:] = K11.reshape(g + (3 * MUL, 3 * MUL))
    kern = np.transpose(kern, (4, 3, 0, 1, 2))  # -> [o, i, kx, ky, kz]

    inv_s8 = 1.0 / math.sqrt(MUL)
    # self-connection -> center tap
    for u in range(MUL):
        for v in range(MUL):
            kern[v, u, 2, 2, 2] += inv_s8 * w_sc0[u, v]
            for m in range(3):
                kern[MUL + 3 * v + m, MUL + 3 * u + m, 2, 2, 2] += (
                    inv_s8 * w_sc1[u, v]
                )
    return kern.astype(np.float32)


def build_lhsT(kern):
    """Pack kern into the 15 [128,128] stationary matrices -> [128, 15*128].

    lhsT_i[a*32 + ci, b*32 + co] = kern[co, ci, dx+2, dy+2, dz+2] for the
    tap realized by block (a, b) of instruction i; each tap assigned once.
    """
    wbuf = np.zeros((128, NI * 128), np.float32)
    assigned = set()
    for i, (dx, ty) in enumerate(INSTRS):
        for bi, (by, bz) in enumerate(W_SET):
            dy = ty - by
            for a in range(4):
                dz = a - bz
                if abs(dy) <= 2 and abs(dz) <= 2 and (dx, dy, dz) not in assigned:
                    assigned.add((dx, dy, dz))
                    wbuf[a * 32:(a + 1) * 32, i * 128 + bi * 32:i * 128 + (bi + 1) * 32] = \
                        kern[:, :, dx + 2, dy + 2, dz + 2].T
    assert len(assigned) == 125
    return wbuf


# ---------------------------------------------------------------------------
# Device program
# ---------------------------------------------------------------------------

_NC_CACHE = {}


def _build_program(repeat=1):
    import concourse.bacc as bacc
    import concourse.mybir as mybir
    import concourse.tile as tile

    f32 = mybir.dt.float32
    f32r = mybir.dt.float32r
    add = mybir.AluOpType.add

    WFLAT = tuple(by * YP + bz for by, bz in W_SET)   # (0, 68, 2, 70)
    GW = NTILE * TILEN + 128 + 72                     # gbuf width
    LASTN = 392                                       # tail tile free size

    nc = bacc.Bacc("TRN2", target_bir_lowering=False, debug=False)
    x_d = nc.dram_tensor("xc", [C, NROW + 4, YP, 72], f32r, kind="ExternalInput")
    w_d = nc.dram_tensor("wt", [128, NI * 128], f32r, kind="ExternalInput")
    c_d = nc.dram_tensor("cw", [128, 32], f32r, kind="ExternalInput")
    o_d = nc.dram_tensor("out", [C, NROW, 64, 64], f32, kind="ExternalOutput")

    with tile.TileContext(nc) as tc:
        with (
            tc.tile_pool(name="xin", bufs=5) as xpool,
            tc.tile_pool(name="wts", bufs=1) as wpool,
            tc.tile_pool(name="gb", bufs=2) as gpool,
            tc.tile_pool(name="orow", bufs=1) as opool,
            tc.tile_pool(name="ps", bufs=6, space="PSUM") as pspool,
            tc.tile_pool(name="ps2", bufs=2, space="PSUM") as ps2pool,
        ):
            wt = wpool.tile([128, NI * 128], f32r)
            nc.sync.dma_start(wt[:], w_d.ap())
            cwt = wpool.tile([128, 32], f32r, name="cwt", tag="cwt")
            nc.sync.dma_start(cwt[:], c_d.ap())
            orow = opool.tile([32, 2 * S], f32)

            def body():
                xin = {}

                def load_row(j):
                    t = xpool.tile([128, TOT], f32r, name="xrow", tag="xrow")
                    nc.gpsimd.memset(t[:, 0:G].bitcast(f32), 0.0)
                    nc.gpsimd.memset(t[:, G + S:TOT].bitcast(f32), 0.0)
                    for a in range(4):
                        nc.sync.dma_start(
                            t[a * 32:(a + 1) * 32, G:G + S].rearrange(
                                "p (y z) -> p y z", y=YP),
                            x_d.ap()[:, j, :, a:a + YP],
                        )
                    xin[j] = t

                for j in range(5):
                    load_row(j)
                for r in range(NROW):
                    if r + 5 < NROW + 4:
                        load_row(r + 5)
                    gb = gpool.tile([128, GW], f32r, name="gbuf", tag="gbuf")
                    nc.gpsimd.memset(gb[:, 0:128].bitcast(f32), 0.0)
                    so = (r % 2) * S
                    for ti in range(NTILE):
                        F0 = ti * TILEN
                        N = LASTN if ti == NTILE - 1 else TILEN
                        po = pspool.tile([128, TILEN], f32, name="po", tag="po")
                        for i, (dx, ty) in enumerate(INSTRS):
                            off = G + F0 + ty * YP
                            nc.tensor.matmul(
                                po[:, 0:N], wt[:, i * 128:(i + 1) * 128],
                                xin[r + 2 + dx][:, off:off + N],
                                start=(i == 0), stop=(i == NI - 1),
                            )
                        for b in range(4):
                            dst = gb[b * 32:(b + 1) * 32,
                                     F0 + WFLAT[b]:F0 + WFLAT[b] + N]
                            src = po[b * 32:(b + 1) * 32, 0:N]
                            if b < 2:
                                nc.scalar.copy(dst, src)
                            else:
                                nc.vector.tensor_copy(dst, src)
                        po2 = ps2pool.tile([32, TILEN], f32, name="po2",
                                           tag="po2")
                        nc.tensor.matmul(
                            po2[:, 0:N], cwt[:], gb[:, F0:F0 + N],
                            start=True, stop=True,
                        )
                        nc.vector.tensor_copy(
                            orow[:, so + F0:so + F0 + N], po2[:, 0:N])
                    nc.sync.dma_start(
                        o_d.ap()[:, r],
                        orow[:, so:so + S].rearrange(
                            "p (y z) -> p y z", y=YP)[:, 2:66, 2:66],
                    )

            if repeat > 1:
                with tc.For_i(0, repeat):
                    body()
            else:
                body()

    nc.compile()
    return nc


def get_program(repeat=1):
    if repeat not in _NC_CACHE:
        _NC_CACHE[repeat] = _build_program(repeat)
    return _NC_CACHE[repeat]


def make_in_maps(x, weight, w_sc0, w_sc1):
    kern = build_folded_kernel(np.asarray(weight), np.asarray(w_sc0),
                               np.asarray(w_sc1))
    wbuf = build_lhsT(kern)
    cw = np.zeros((128, 32), np.float32)
    for b in range(4):
        cw[b * 32:(b + 1) * 32, :] = np.eye(32, dtype=np.float32)
    B = x.shape[0]
    xpad = np.zeros((B, C, 68, 68, 72), np.float32)
    xpad[:, :, 2:66, 2:66, 2:66] = x
    in_maps = []
    for core in range(8):
        n, xi = divmod(core, 4)
        xc = np.ascontiguousarray(xpad[n, :, 16 * xi:16 * xi + 20])
        in_maps.append({"xc": xc, "wt": wbuf, "cw": cw})
    return in_maps


def kernel(x, weight, w_sc0, w_sc1):
    from concourse import bass2jax

    x = np.asarray(x)
    nc = get_program(1)
    in_maps = make_in_maps(x, weight, w_sc0, w_sc1)
    results = bass2jax.run_bass_via_pjrt(nc, in_maps, n_cores=8)
    out = np.empty((2, C, 64, 64, 64), np.float32)
    for core in range(8):
        n, xi = divmod(core, 4)
        out[n, :, 16 * xi:16 * xi + 16] = results[core]["out"]
    return out
